# revision 10
# baseline (speedup 1.0000x reference)
"""DenoisingAttention (NVIB non-meta path) Trainium2 Bass kernel.

Data-parallel over the batch dim: core b computes batch element b end-to-end
(B == 8 == n_cores, no collectives).

Layout strategy: every matmul on the PE contracts over the partition dim, so
all operands are staged with the contraction dim on partitions.  The host
passes pre-transposed copies of the inputs (x^T, mu^T, logvar^T, W^T) so the
kernel needs no on-chip transposes; outputs are produced transposed
(out^T = (C, N), attn^T = (H, keys, queries)) and the host transposes back.

Per-core pipeline:
  1. elementwise: rbv = 1/(exp(logvar^T)+sqrt(hd)); kin^T = mu^T*rbv;
     prod^T = mu^T*kin^T
  2. l2[key] = sum_c 0.5*prod^T  via fp32 ones-matmuls (N=1); softmax key bias
     bias[key] = ln(alpha[key]) - l2[key]
  3. projections (fp32r): q^T = (w_q^T)^T-matmuls, k^T likewise from kin^T,
     v natural layout from kin^T as stationary operand (w_v pre-scaled by
     sqrt(hd) on host)
  4. per head: logits^T = k_h^T x q_h^T (keys x queries), exp+bias fused on
     ScalarE, denominators via ones-matmul, mm2 attn^T-weighted v, normalize
  5. final projection out^T = w_p x attn_out^T + b_p
"""

import os
import sys
import threading

for _p in ("/opt/trn_rl_repo",):
    if _p not in sys.path and os.path.isdir(_p):
        sys.path.insert(0, _p)

import numpy as np

import concourse.bass as bass
import concourse.bacc as bacc
import concourse.mybir as mybir
import concourse.tile as tile

B, N, C, H, HD = 8, 512, 1024, 16, 64
P = 128
NP = C // P        # 8 feature-dim partition tiles
NT = N // P        # 4 token-dim partition tiles
NH2 = H // 2       # head pairs (two heads share one 128-partition tile)
SQRT_HD = 8.0      # sqrt(HD)
F32 = mybir.dt.float32
F32R = mybir.dt.float32r
AF = mybir.ActivationFunctionType
N_CORES = 8


def build_bass():
    nc = bacc.Bacc("TRN2", target_bir_lowering=False, debug=False,
                   num_devices=N_CORES)

    xT = nc.dram_tensor("xT", [C, N], F32R, kind="ExternalInput").ap()
    muT = nc.dram_tensor("muT", [C, N], F32, kind="ExternalInput").ap()
    lvT = nc.dram_tensor("lvT", [C, N], F32, kind="ExternalInput").ap()
    al = nc.dram_tensor("alpha_r", [P, NT], F32, kind="ExternalInput").ap()
    wq = nc.dram_tensor("wqT", [C, C], F32R, kind="ExternalInput").ap()
    wk = nc.dram_tensor("wkT", [C, C], F32R, kind="ExternalInput").ap()
    wv = nc.dram_tensor("wvT8", [C, C], F32R, kind="ExternalInput").ap()
    wp = nc.dram_tensor("wpT", [C, C], F32R, kind="ExternalInput").ap()
    bp = nc.dram_tensor("bp_r", [P, NP], F32, kind="ExternalInput").ap()
    attn_o = nc.dram_tensor("attn_t", [H, N, N], F32, kind="ExternalOutput").ap()
    out_o = nc.dram_tensor("out_t", [C, N], F32, kind="ExternalOutput").ap()

    with tile.TileContext(nc) as tc:
        with (
            tc.tile_pool(name="persist", bufs=1) as pp,
            tc.tile_pool(name="acts", bufs=1) as ac,
            tc.tile_pool(name="wpool", bufs=3) as wpl,
            tc.tile_pool(name="expp", bufs=8) as ep,
            tc.tile_pool(name="attn_sb", bufs=4) as asb,
            tc.tile_pool(name="bcast", bufs=4) as bcp,
            tc.tile_pool(name="small", bufs=4) as sp,
            tc.tile_pool(name="fin", bufs=3) as fpl,
            tc.tile_pool(name="ps", bufs=8, space="PSUM") as ps,
        ):
            # constants + per-key bias inputs
            ones_f = pp.tile([P, 1], F32, tag="ones_f")
            nc.vector.memset(ones_f, 1.0)
            ones = pp.tile([P, 1], F32R, tag="ones")
            nc.vector.tensor_copy(ones, ones_f)
            ones_row_f = pp.tile([1, P], F32, tag="ones_row_f")
            nc.vector.memset(ones_row_f, 1.0)
            ones_row = pp.tile([1, P], F32R, tag="ones_row")
            nc.vector.tensor_copy(ones_row, ones_row_f)
            halves = pp.tile([P, 1], F32, tag="halves")
            nc.vector.memset(halves, 0.5)
            alpha_t = pp.tile([P, NT], F32, tag="alpha")
            nc.sync.dma_start(out=alpha_t, in_=al)
            lal = pp.tile([P, NT], F32, tag="lal")
            nc.scalar.activation(lal, alpha_t, AF.Ln)
            bias4 = pp.tile([P, NT], F32, tag="bias4")
            bp_t = pp.tile([P, NP], F32, tag="bp")
            nc.sync.dma_start(out=bp_t, in_=bp)

            # ---- phase E: elementwise NVIB transforms (transposed layout) ----
            mu_t, kin_t, x_t = [], [], []
            for i in range(NP):
                sl = slice(i * P, (i + 1) * P)
                lv = ac.tile([P, N], F32, tag=f"lv{i}")
                nc.sync.dma_start(out=lv, in_=lvT[sl, :])
                mu = ac.tile([P, N], F32, tag=f"mu{i}")
                nc.sync.dma_start(out=mu, in_=muT[sl, :])
                x = ac.tile([P, N], F32R, tag=f"x{i}")
                nc.sync.dma_start(out=x, in_=xT[sl, :])
                # lv <- 1 / (exp(lv) + sqrt_hd)   (biased variance reciprocal)
                nc.scalar.activation(lv, lv, AF.Exp)
                nc.vector.tensor_scalar_add(lv, lv, SQRT_HD)
                nc.vector.reciprocal(lv, lv)
                kin = ac.tile([P, N], F32R, tag=f"kin{i}")
                nc.vector.tensor_mul(kin, mu, lv)
                # mu <- mu * kin = mu^2 / biased_var   (l2 integrand)
                nc.vector.tensor_mul(mu, mu, kin.bitcast(F32))
                mu_t.append(mu)
                kin_t.append(kin)
                x_t.append(x)

            # ---- l2 per key + softmax key bias (full fp32 matmuls, N=1) ----
            for kc in range(NT):
                pl2 = ps.tile([P, 1], F32, tag="ps")
                for i in range(NP):
                    nc.tensor.matmul(pl2, mu_t[i][:, kc * P:(kc + 1) * P], halves,
                                     start=(i == 0), stop=(i == NP - 1))
                nc.vector.tensor_sub(bias4[:, kc:kc + 1], lal[:, kc:kc + 1], pl2)

            # ---- projections ----
            def project_T(wdram, rhs_tiles, name):
                """out^T tiles (c_out on partitions): lhsT = W^T k-slices."""
                psums = [ps.tile([P, N], F32, tag="ps", name=f"psum_{name}{m}")
                         for m in range(NP)]
                for k in range(NP):
                    wt = wpl.tile([P, C], F32R, tag="w")
                    nc.sync.dma_start(out=wt, in_=wdram[k * P:(k + 1) * P, :])
                    for m in range(NP):
                        nc.tensor.matmul(
                            psums[m],
                            wt[:, m * P:(m + 1) * P],
                            rhs_tiles[k],
                            start=(k == 0), stop=(k == NP - 1))
                outs = []
                for m in range(NP):
                    o = pp.tile([P, N], F32R, tag=f"{name}{m}")
                    nc.vector.tensor_copy(o, psums[m])
                    outs.append(o)
                return outs

            qT_t = project_T(wq, x_t, "qT")
            kT_t = project_T(wk, kin_t, "kT")

            # v in natural layout (tokens on partitions): lhsT = kin^T slices
            v_t = [pp.tile([P, C], F32R, tag=f"v{t}", name=f"v{t}")
                   for t in range(NT)]
            psv = [[ps.tile([P, N], F32, tag="ps", name=f"psv{t}_{jj}")
                    for jj in range(2)] for t in range(NT)]
            for k in range(NP):
                wt = wpl.tile([P, C], F32R, tag="w")
                nc.sync.dma_start(out=wt, in_=wv[k * P:(k + 1) * P, :])
                for t in range(NT):
                    for j in range(2):
                        nc.tensor.matmul(
                            psv[t][j],
                            kin_t[k][:, t * P:(t + 1) * P],
                            wt[:, j * N:(j + 1) * N],
                            start=(k == 0), stop=(k == NP - 1))
            for t in range(NT):
                for j in range(2):
                    nc.vector.tensor_copy(v_t[t][:, j * N:(j + 1) * N], psv[t][j])

            # ---- attention, one head pair per 128-partition tile ----
            out_t_tiles = []
            for j in range(NH2):
                exp_tiles = [[None] * NT, [None] * NT]
                bc = [None, None]
                ps_o = [None, None]
                # mm1 both heads first so PE stays busy while ACT does exps
                for e in range(2):
                    off = 64 * e
                    qh = qT_t[j][off:off + 64, :]
                    for kc in range(NT):
                        psl = ps.tile([P, N], F32, tag="ps",
                                      name=f"psl{j}_{e}_{kc}")
                        nc.tensor.matmul(
                            psl,
                            kT_t[j][off:off + 64, kc * P:(kc + 1) * P],
                            qh, start=True, stop=True)
                        et = ep.tile([P, N], F32R, tag="exp",
                                     name=f"et{j}_{e}_{kc}")
                        nc.scalar.activation(et, psl, AF.Exp,
                                             bias=bias4[:, kc:kc + 1], scale=1.0)
                        exp_tiles[e][kc] = et
                for e in range(2):
                    h = 2 * j + e
                    ps_d = ps.tile([1, N], F32, tag="ps", name=f"psd{j}_{e}")
                    for kc in range(NT):
                        nc.tensor.matmul(ps_d, ones,
                                         exp_tiles[e][kc],
                                         start=(kc == 0), stop=(kc == NT - 1))
                    po = ps.tile([64, N], F32, tag="ps", name=f"pso{j}_{e}")
                    for kc in range(NT):
                        nc.tensor.matmul(
                            po,
                            v_t[kc][:, h * HD:(h + 1) * HD],
                            exp_tiles[e][kc],
                            start=(kc == 0), stop=(kc == NT - 1))
                    ps_o[e] = po
                    rec = sp.tile([1, N], F32R, tag="rec", name=f"rec{j}_{e}")
                    with nc.allow_low_precision(reason="fp32r for PE broadcast"):
                        nc.vector.reciprocal(rec, ps_d)
                    # broadcast recip row to 128 partitions: ones_row outer rec
                    bc_ps = ps.tile([P, N], F32, tag="ps", name=f"bcps{j}_{e}")
                    nc.tensor.matmul(bc_ps, ones_row, rec, start=True, stop=True)
                    bc[e] = bcp.tile([P, N], F32, tag="bc", name=f"bc{j}_{e}")
                    nc.vector.tensor_copy(bc[e], bc_ps)
                    for kc in range(NT):
                        at = asb.tile([P, N], F32, tag="at",
                                      name=f"at{j}_{e}_{kc}")
                        nc.vector.tensor_mul(at, exp_tiles[e][kc].bitcast(F32),
                                             bc[e])
                        nc.sync.dma_start(
                            out=attn_o[h, kc * P:(kc + 1) * P, :], in_=at)
                ot = pp.tile([P, N], F32R, tag=f"ot{j}")
                nc.vector.tensor_mul(ot[0:64, :], ps_o[0][:, :], bc[0][0:64, :])
                oth = sp.tile([64, N], F32R, tag="oth", name=f"oth{j}")
                nc.vector.tensor_mul(oth, ps_o[1][:, :], bc[1][0:64, :])
                nc.sync.dma_start(out=ot[64:128, :], in_=oth)
                out_t_tiles.append(ot)

            # ---- final projection out^T = w_p @ attn_out^T + b_p ----
            psf = [ps.tile([P, N], F32, tag="ps", name=f"psf{m}")
                   for m in range(NP)]
            for k in range(NP):
                wt = wpl.tile([P, C], F32R, tag="w")
                nc.sync.dma_start(out=wt, in_=wp[k * P:(k + 1) * P, :])
                for m in range(NP):
                    # rhs: k-th 128-row slice of attn_out^T == head-pair tile k
                    nc.tensor.matmul(
                        psf[m],
                        wt[:, m * P:(m + 1) * P],
                        out_t_tiles[k],
                        start=(k == 0), stop=(k == NP - 1))
            for m in range(NP):
                fo = fpl.tile([P, N], F32, tag="fo")
                nc.vector.tensor_scalar_add(fo, psf[m], bp_t[:, m:m + 1])
                nc.sync.dma_start(out=out_o[m * P:(m + 1) * P, :], in_=fo)

    nc.compile()
    return nc


def shard_inputs(x, mu, logvar, alpha, w_q, w_k, w_v, w_p, b_p):
    """Host-side data prep: transpose + reshape, one in_map per core."""
    f = np.float32
    wqT = np.ascontiguousarray(np.asarray(w_q, f).T)
    wkT = np.ascontiguousarray(np.asarray(w_k, f).T)
    wvT8 = np.ascontiguousarray(np.asarray(w_v, f).T * SQRT_HD)
    wpT = np.ascontiguousarray(np.asarray(w_p, f).T)
    bp_r = np.ascontiguousarray(np.asarray(b_p, f).reshape(NP, P).T)
    in_maps = []
    for b in range(B):
        a = np.asarray(alpha[b], f).reshape(N)
        in_maps.append({
            "xT": np.ascontiguousarray(np.asarray(x[b], f).T),
            "muT": np.ascontiguousarray(np.asarray(mu[b], f).T),
            "lvT": np.ascontiguousarray(np.asarray(logvar[b], f).T),
            "alpha_r": np.ascontiguousarray(a.reshape(NT, P).T),
            "wqT": wqT, "wkT": wkT, "wvT8": wvT8, "wpT": wpT, "bp_r": bp_r,
        })
    return in_maps


def unshard_outputs(results):
    """results: list of per-core {attn_t, out_t} -> (out, attn) full arrays."""
    out = np.empty((B, N, C), np.float32)
    attn = np.empty((B, H, N, N), np.float32)
    for b, r in enumerate(results):
        out[b] = r["out_t"].T
        attn[b] = r["attn_t"].transpose(0, 2, 1)
    return out, attn


_RUNNER_LOCK = threading.Lock()
_RUNNER = None


def _get_runner():
    """Build the Bass module + jitted SPMD executable once; reuse across calls."""
    global _RUNNER
    with _RUNNER_LOCK:
        if _RUNNER is not None:
            return _RUNNER

        nc = build_bass()

        import jax
        from jax.sharding import Mesh, PartitionSpec
        from jax.experimental.shard_map import shard_map
        from concourse import bass2jax
        import concourse.mybir as mb

        bass2jax.install_neuronx_cc_hook()

        partition_name = (nc.partition_id_tensor.name
                          if nc.partition_id_tensor else None)
        in_names, out_names, out_avals, zero_shapes = [], [], [], []
        for alloc in nc.m.functions[0].allocations:
            if not isinstance(alloc, mb.MemoryLocationSet):
                continue
            name = alloc.memorylocations[0].name
            if alloc.kind == "ExternalInput":
                if name != partition_name:
                    in_names.append(name)
            elif alloc.kind == "ExternalOutput":
                shape = tuple(alloc.tensor_shape)
                dtype = mb.dt.np(alloc.dtype)
                out_names.append(name)
                out_avals.append(jax.core.ShapedArray(shape, dtype))
                zero_shapes.append((shape, dtype))
        n_params = len(in_names)
        all_names = in_names + out_names
        if partition_name is not None:
            all_names.append(partition_name)
        donate = tuple(range(n_params, n_params + len(out_names)))

        def _body(*args):
            operands = list(args)
            if partition_name is not None:
                operands.append(bass2jax.partition_id_tensor())
            outs = bass2jax._bass_exec_p.bind(
                *operands,
                out_avals=tuple(out_avals),
                in_names=tuple(all_names),
                out_names=tuple(out_names),
                lowering_input_output_aliases=(),
                sim_require_finite=True,
                sim_require_nnan=True,
                nc=nc,
            )
            return tuple(outs)

        devices = jax.devices()[:N_CORES]
        mesh = Mesh(np.asarray(devices), ("core",))
        in_specs = (PartitionSpec("core"),) * (n_params + len(out_names))
        out_specs = (PartitionSpec("core"),) * len(out_names)
        sharded = jax.jit(
            shard_map(_body, mesh=mesh, in_specs=in_specs,
                      out_specs=out_specs, check_rep=False),
            donate_argnums=donate, keep_unused=True)

        def execute(in_maps):
            per_core = [[np.asarray(m[n]) for n in in_names] for m in in_maps]
            concat_in = [
                np.concatenate([per_core[c][i] for c in range(N_CORES)], axis=0)
                for i in range(n_params)
            ]
            concat_zeros = [
                np.zeros((N_CORES * s[0], *s[1:]), d) for (s, d) in zero_shapes
            ]
            out_arrs = sharded(*concat_in, *concat_zeros)
            out_arrs = [np.asarray(a) for a in out_arrs]
            return [
                {name: out_arrs[i].reshape(N_CORES, *zero_shapes[i][0])[c]
                 for i, name in enumerate(out_names)}
                for c in range(N_CORES)
            ]

        _RUNNER = execute
        return _RUNNER


def kernel(x, mu, logvar, alpha, pi, z, w_q, w_k, w_v, w_p, b_p):
    in_maps = shard_inputs(x, mu, logvar, alpha, w_q, w_k, w_v, w_p, b_p)
    results = _get_runner()(in_maps)
    return unshard_outputs(results)


# revision 11
# speedup vs baseline: 656.7191x; 656.7191x over previous
"""DenoisingAttention (NVIB non-meta path) Trainium2 Bass kernel.

Data-parallel over the batch dim: core b computes batch element b end-to-end
(B == 8 == n_cores, no collectives).

Layout strategy: every matmul on the PE contracts over the partition dim, so
all operands are staged with the contraction dim on partitions.  The host
passes pre-transposed copies of the inputs (x^T, mu^T, logvar^T, W^T) so the
kernel needs no on-chip transposes; outputs are produced transposed
(out^T = (C, N), attn^T = (H, keys, queries)) and the host transposes back.

Per-core pipeline:
  1. elementwise: rbv = 1/(exp(logvar^T)+sqrt(hd)); kin^T = mu^T*rbv;
     prod^T = mu^T*kin^T
  2. l2[key] = sum_c 0.5*prod^T  via fp32 ones-matmuls (N=1); softmax key bias
     bias[key] = ln(alpha[key]) - l2[key]
  3. projections (fp32r): q^T = (w_q^T)^T-matmuls, k^T likewise from kin^T,
     v natural layout from kin^T as stationary operand (w_v pre-scaled by
     sqrt(hd) on host)
  4. per head: logits^T = k_h^T x q_h^T (keys x queries), exp+bias fused on
     ScalarE, denominators via ones-matmul, mm2 attn^T-weighted v, normalize
  5. final projection out^T = w_p x attn_out^T + b_p
"""

import os
import sys
import threading

for _p in ("/opt/trn_rl_repo",):
    if _p not in sys.path and os.path.isdir(_p):
        sys.path.insert(0, _p)

import numpy as np

import concourse.bass as bass
import concourse.bacc as bacc
import concourse.mybir as mybir
import concourse.tile as tile

B, N, C, H, HD = 8, 512, 1024, 16, 64
P = 128
NP = C // P        # 8 feature-dim partition tiles
NT = N // P        # 4 token-dim partition tiles
NH2 = H // 2       # head pairs (two heads share one 128-partition tile)
SQRT_HD = 8.0      # sqrt(HD)
F32 = mybir.dt.float32
F32R = mybir.dt.float32r
AF = mybir.ActivationFunctionType
N_CORES = 8


def build_bass():
    nc = bacc.Bacc("TRN2", target_bir_lowering=False, debug=False,
                   num_devices=N_CORES)

    xT = nc.dram_tensor("xT", [C, N], F32R, kind="ExternalInput").ap()
    muT = nc.dram_tensor("muT", [C, N], F32, kind="ExternalInput").ap()
    lvT = nc.dram_tensor("lvT", [C, N], F32, kind="ExternalInput").ap()
    al = nc.dram_tensor("alpha_r", [P, NT], F32, kind="ExternalInput").ap()
    wq = nc.dram_tensor("wqT", [C, C], F32R, kind="ExternalInput").ap()
    wk = nc.dram_tensor("wkT", [C, C], F32R, kind="ExternalInput").ap()
    wv = nc.dram_tensor("wvT8", [C, C], F32R, kind="ExternalInput").ap()
    wp = nc.dram_tensor("wpT", [C, C], F32R, kind="ExternalInput").ap()
    bp = nc.dram_tensor("bp_r", [P, NP], F32, kind="ExternalInput").ap()
    attn_o = nc.dram_tensor("attn_t", [H, N, N], F32, kind="ExternalOutput").ap()
    out_o = nc.dram_tensor("out_t", [C, N], F32, kind="ExternalOutput").ap()

    with tile.TileContext(nc) as tc:
        with (
            tc.tile_pool(name="persist", bufs=1) as pp,
            tc.tile_pool(name="acts", bufs=1) as ac,
            tc.tile_pool(name="wpool", bufs=3) as wpl,
            tc.tile_pool(name="expp", bufs=8) as ep,
            tc.tile_pool(name="attn_sb", bufs=4) as asb,
            tc.tile_pool(name="bcast", bufs=4) as bcp,
            tc.tile_pool(name="small", bufs=4) as sp,
            tc.tile_pool(name="fin", bufs=3) as fpl,
            tc.tile_pool(name="ps", bufs=8, space="PSUM") as ps,
        ):
            # constants + per-key bias inputs
            ones_f = pp.tile([P, 1], F32, tag="ones_f")
            nc.vector.memset(ones_f, 1.0)
            ones = pp.tile([P, 1], F32R, tag="ones")
            nc.vector.tensor_copy(ones, ones_f)
            ones_row_f = pp.tile([1, P], F32, tag="ones_row_f")
            nc.vector.memset(ones_row_f, 1.0)
            ones_row = pp.tile([1, P], F32R, tag="ones_row")
            nc.vector.tensor_copy(ones_row, ones_row_f)
            halves = pp.tile([P, 1], F32, tag="halves")
            nc.vector.memset(halves, 0.5)
            alpha_t = pp.tile([P, NT], F32, tag="alpha")
            nc.sync.dma_start(out=alpha_t, in_=al)
            lal = pp.tile([P, NT], F32, tag="lal")
            nc.scalar.activation(lal, alpha_t, AF.Ln)
            bias4 = pp.tile([P, NT], F32, tag="bias4")
            bp_t = pp.tile([P, NP], F32, tag="bp")
            nc.sync.dma_start(out=bp_t, in_=bp)

            # ---- phase E: elementwise NVIB transforms (transposed layout) ----
            mu_t, kin_t, x_t = [], [], []
            for i in range(NP):
                sl = slice(i * P, (i + 1) * P)
                lv = ac.tile([P, N], F32, tag=f"lv{i}")
                nc.sync.dma_start(out=lv, in_=lvT[sl, :])
                mu = ac.tile([P, N], F32, tag=f"mu{i}")
                nc.sync.dma_start(out=mu, in_=muT[sl, :])
                x = ac.tile([P, N], F32R, tag=f"x{i}")
                nc.sync.dma_start(out=x, in_=xT[sl, :])
                # lv <- 1 / (exp(lv) + sqrt_hd)   (biased variance reciprocal)
                nc.scalar.activation(lv, lv, AF.Exp)
                nc.vector.tensor_scalar_add(lv, lv, SQRT_HD)
                nc.vector.reciprocal(lv, lv)
                kin = ac.tile([P, N], F32R, tag=f"kin{i}")
                nc.vector.tensor_mul(kin, mu, lv)
                # mu <- mu * kin = mu^2 / biased_var   (l2 integrand)
                nc.vector.tensor_mul(mu, mu, kin.bitcast(F32))
                mu_t.append(mu)
                kin_t.append(kin)
                x_t.append(x)

            # ---- l2 per key + softmax key bias (full fp32 matmuls, N=1) ----
            for kc in range(NT):
                pl2 = ps.tile([P, 1], F32, tag="ps")
                for i in range(NP):
                    nc.tensor.matmul(pl2, mu_t[i][:, kc * P:(kc + 1) * P], halves,
                                     start=(i == 0), stop=(i == NP - 1))
                nc.vector.tensor_sub(bias4[:, kc:kc + 1], lal[:, kc:kc + 1], pl2)

            # ---- projections ----
            def project_T(wdram, rhs_tiles, name):
                """out^T tiles (c_out on partitions): lhsT = W^T k-slices."""
                psums = [ps.tile([P, N], F32, tag="ps", name=f"psum_{name}{m}")
                         for m in range(NP)]
                for k in range(NP):
                    wt = wpl.tile([P, C], F32R, tag="w")
                    nc.sync.dma_start(out=wt, in_=wdram[k * P:(k + 1) * P, :])
                    for m in range(NP):
                        nc.tensor.matmul(
                            psums[m],
                            wt[:, m * P:(m + 1) * P],
                            rhs_tiles[k],
                            start=(k == 0), stop=(k == NP - 1))
                outs = []
                for m in range(NP):
                    o = pp.tile([P, N], F32R, tag=f"{name}{m}")
                    nc.vector.tensor_copy(o, psums[m])
                    outs.append(o)
                return outs

            qT_t = project_T(wq, x_t, "qT")
            kT_t = project_T(wk, kin_t, "kT")

            # v in natural layout (tokens on partitions): lhsT = kin^T slices
            v_t = [pp.tile([P, C], F32R, tag=f"v{t}", name=f"v{t}")
                   for t in range(NT)]
            psv = [[ps.tile([P, N], F32, tag="ps", name=f"psv{t}_{jj}")
                    for jj in range(2)] for t in range(NT)]
            for k in range(NP):
                wt = wpl.tile([P, C], F32R, tag="w")
                nc.sync.dma_start(out=wt, in_=wv[k * P:(k + 1) * P, :])
                for t in range(NT):
                    for j in range(2):
                        nc.tensor.matmul(
                            psv[t][j],
                            kin_t[k][:, t * P:(t + 1) * P],
                            wt[:, j * N:(j + 1) * N],
                            start=(k == 0), stop=(k == NP - 1))
            for t in range(NT):
                for j in range(2):
                    nc.vector.tensor_copy(v_t[t][:, j * N:(j + 1) * N], psv[t][j])

            # ---- attention, one head pair per 128-partition tile ----
            out_t_tiles = []
            for j in range(NH2):
                exp_tiles = [[None] * NT, [None] * NT]
                bc = [None, None]
                ps_o = [None, None]
                # mm1 both heads first so PE stays busy while ACT does exps
                for e in range(2):
                    off = 64 * e
                    qh = qT_t[j][off:off + 64, :]
                    for kc in range(NT):
                        psl = ps.tile([P, N], F32, tag="ps",
                                      name=f"psl{j}_{e}_{kc}")
                        nc.tensor.matmul(
                            psl,
                            kT_t[j][off:off + 64, kc * P:(kc + 1) * P],
                            qh, start=True, stop=True)
                        et = ep.tile([P, N], F32R, tag="exp",
                                     name=f"et{j}_{e}_{kc}")
                        nc.scalar.activation(et, psl, AF.Exp,
                                             bias=bias4[:, kc:kc + 1], scale=1.0)
                        exp_tiles[e][kc] = et
                for e in range(2):
                    h = 2 * j + e
                    ps_d = ps.tile([1, N], F32, tag="ps", name=f"psd{j}_{e}")
                    for kc in range(NT):
                        nc.tensor.matmul(ps_d, ones,
                                         exp_tiles[e][kc],
                                         start=(kc == 0), stop=(kc == NT - 1))
                    po = ps.tile([64, N], F32, tag="ps", name=f"pso{j}_{e}")
                    for kc in range(NT):
                        nc.tensor.matmul(
                            po,
                            v_t[kc][:, h * HD:(h + 1) * HD],
                            exp_tiles[e][kc],
                            start=(kc == 0), stop=(kc == NT - 1))
                    ps_o[e] = po
                    rec = sp.tile([1, N], F32R, tag="rec", name=f"rec{j}_{e}")
                    with nc.allow_low_precision(reason="fp32r for PE broadcast"):
                        nc.vector.reciprocal(rec, ps_d)
                    # broadcast recip row to 128 partitions: ones_row outer rec
                    bc_ps = ps.tile([P, N], F32, tag="ps", name=f"bcps{j}_{e}")
                    nc.tensor.matmul(bc_ps, ones_row, rec, start=True, stop=True)
                    bc[e] = bcp.tile([P, N], F32, tag="bc", name=f"bc{j}_{e}")
                    nc.vector.tensor_copy(bc[e], bc_ps)
                    for kc in range(NT):
                        at = asb.tile([P, N], F32, tag="at",
                                      name=f"at{j}_{e}_{kc}")
                        nc.vector.tensor_mul(at, exp_tiles[e][kc].bitcast(F32),
                                             bc[e])
                        nc.sync.dma_start(
                            out=attn_o[h, kc * P:(kc + 1) * P, :], in_=at)
                ot = pp.tile([P, N], F32R, tag=f"ot{j}")
                nc.vector.tensor_mul(ot[0:64, :], ps_o[0][:, :], bc[0][0:64, :])
                oth = sp.tile([64, N], F32R, tag="oth", name=f"oth{j}")
                nc.vector.tensor_mul(oth, ps_o[1][:, :], bc[1][0:64, :])
                nc.sync.dma_start(out=ot[64:128, :], in_=oth)
                out_t_tiles.append(ot)

            # ---- final projection out^T = w_p @ attn_out^T + b_p ----
            psf = [ps.tile([P, N], F32, tag="ps", name=f"psf{m}")
                   for m in range(NP)]
            for k in range(NP):
                wt = wpl.tile([P, C], F32R, tag="w")
                nc.sync.dma_start(out=wt, in_=wp[k * P:(k + 1) * P, :])
                for m in range(NP):
                    # rhs: k-th 128-row slice of attn_out^T == head-pair tile k
                    nc.tensor.matmul(
                        psf[m],
                        wt[:, m * P:(m + 1) * P],
                        out_t_tiles[k],
                        start=(k == 0), stop=(k == NP - 1))
            for m in range(NP):
                fo = fpl.tile([P, N], F32, tag="fo")
                nc.vector.tensor_scalar_add(fo, psf[m], bp_t[:, m:m + 1])
                nc.sync.dma_start(out=out_o[m * P:(m + 1) * P, :], in_=fo)

    nc.compile()
    return nc


def shard_inputs(x, mu, logvar, alpha, w_q, w_k, w_v, w_p, b_p):
    """Host-side data prep: transpose + reshape, one in_map per core."""
    f = np.float32
    wqT = np.ascontiguousarray(np.asarray(w_q, f).T)
    wkT = np.ascontiguousarray(np.asarray(w_k, f).T)
    wvT8 = np.ascontiguousarray(np.asarray(w_v, f).T * SQRT_HD)
    wpT = np.ascontiguousarray(np.asarray(w_p, f).T)
    bp_r = np.ascontiguousarray(np.asarray(b_p, f).reshape(NP, P).T)
    in_maps = []
    for b in range(B):
        a = np.asarray(alpha[b], f).reshape(N)
        in_maps.append({
            "xT": np.ascontiguousarray(np.asarray(x[b], f).T),
            "muT": np.ascontiguousarray(np.asarray(mu[b], f).T),
            "lvT": np.ascontiguousarray(np.asarray(logvar[b], f).T),
            "alpha_r": np.ascontiguousarray(a.reshape(NT, P).T),
            "wqT": wqT, "wkT": wkT, "wvT8": wvT8, "wpT": wpT, "bp_r": bp_r,
        })
    return in_maps


def unshard_outputs(results):
    """results: list of per-core {attn_t, out_t} -> (out, attn) full arrays."""
    out = np.empty((B, N, C), np.float32)
    attn = np.empty((B, H, N, N), np.float32)
    for b, r in enumerate(results):
        out[b] = r["out_t"].T
        attn[b] = r["attn_t"].transpose(0, 2, 1)
    return out, attn


_RUNNER_LOCK = threading.Lock()
_RUNNER = None


def _get_runner():
    """Build the Bass module + jitted SPMD executable once; reuse across calls."""
    global _RUNNER
    with _RUNNER_LOCK:
        if _RUNNER is not None:
            return _RUNNER

        nc = build_bass()

        import jax
        from jax.sharding import Mesh, PartitionSpec
        from jax.experimental.shard_map import shard_map
        from concourse import bass2jax
        import concourse.mybir as mb

        bass2jax.install_neuronx_cc_hook()

        partition_name = (nc.partition_id_tensor.name
                          if nc.partition_id_tensor else None)
        in_names, out_names, out_avals, zero_shapes = [], [], [], []
        for alloc in nc.m.functions[0].allocations:
            if not isinstance(alloc, mb.MemoryLocationSet):
                continue
            name = alloc.memorylocations[0].name
            if alloc.kind == "ExternalInput":
                if name != partition_name:
                    in_names.append(name)
            elif alloc.kind == "ExternalOutput":
                shape = tuple(alloc.tensor_shape)
                dtype = mb.dt.np(alloc.dtype)
                out_names.append(name)
                out_avals.append(jax.core.ShapedArray(shape, dtype))
                zero_shapes.append((shape, dtype))
        n_params = len(in_names)
        all_names = in_names + out_names
        if partition_name is not None:
            all_names.append(partition_name)
        donate = tuple(range(n_params, n_params + len(out_names)))

        def _body(*args):
            operands = list(args)
            if partition_name is not None:
                operands.append(bass2jax.partition_id_tensor())
            outs = bass2jax._bass_exec_p.bind(
                *operands,
                out_avals=tuple(out_avals),
                in_names=tuple(all_names),
                out_names=tuple(out_names),
                lowering_input_output_aliases=(),
                sim_require_finite=True,
                sim_require_nnan=True,
                nc=nc,
            )
            return tuple(outs)

        devices = jax.devices()[:N_CORES]
        mesh = Mesh(np.asarray(devices), ("core",))
        in_specs = (PartitionSpec("core"),) * (n_params + len(out_names))
        out_specs = (PartitionSpec("core"),) * len(out_names)
        sharded = jax.jit(
            shard_map(_body, mesh=mesh, in_specs=in_specs,
                      out_specs=out_specs, check_rep=False),
            donate_argnums=donate, keep_unused=True)

        class Runner:
            def concat_inputs(self, in_maps):
                per_core = [[np.asarray(m[n]) for n in in_names] for m in in_maps]
                return [
                    np.concatenate([per_core[c][i] for c in range(N_CORES)],
                                   axis=0)
                    for i in range(n_params)
                ]

            def concat_zeros(self):
                return [np.zeros((N_CORES * s[0], *s[1:]), d)
                        for (s, d) in zero_shapes]

            def split_outputs(self, out_arrs):
                out_arrs = [np.asarray(a) for a in out_arrs]
                return [
                    {name: out_arrs[i].reshape(N_CORES, *zero_shapes[i][0])[c]
                     for i, name in enumerate(out_names)}
                    for c in range(N_CORES)
                ]

            def execute(self, in_maps):
                out_arrs = sharded(*self.concat_inputs(in_maps),
                                   *self.concat_zeros())
                return self.split_outputs(out_arrs)

        r = Runner()
        r.sharded = sharded
        r.mesh = mesh
        r.in_names = in_names
        r.out_names = out_names
        r.zero_shapes = zero_shapes
        _RUNNER = r
        return _RUNNER


def kernel(x, mu, logvar, alpha, pi, z, w_q, w_k, w_v, w_p, b_p):
    in_maps = shard_inputs(x, mu, logvar, alpha, w_q, w_k, w_v, w_p, b_p)
    results = _get_runner().execute(in_maps)
    return unshard_outputs(results)


# revision 13
# speedup vs baseline: 2702.4266x; 4.1150x over previous
"""DenoisingAttention (NVIB non-meta path) Trainium2 Bass kernel.

Data-parallel over the batch dim: core b computes batch element b end-to-end
(B == 8 == n_cores, no collectives).

Layout strategy: every matmul on the PE contracts over the partition dim, so
all operands are staged with the contraction dim on partitions.  The host
passes pre-transposed copies of the inputs (x^T, mu^T, logvar^T, W^T) so the
kernel needs no on-chip transposes; outputs are produced transposed
(out^T = (C, N), attn^T = (H, keys, queries)) and the host transposes back.

Per-core pipeline:
  1. elementwise: rbv = 1/(exp(logvar^T)+sqrt(hd)); kin^T = mu^T*rbv;
     prod^T = mu^T*kin^T
  2. l2[key] = sum_c 0.5*prod^T  via fp32 ones-matmuls (N=1); softmax key bias
     bias[key] = ln(alpha[key]) - l2[key]
  3. projections (fp32r): q^T = (w_q^T)^T-matmuls, k^T likewise from kin^T,
     v natural layout from kin^T as stationary operand (w_v pre-scaled by
     sqrt(hd) on host)
  4. per head: logits^T = k_h^T x q_h^T (keys x queries), exp+bias fused on
     ScalarE, denominators via ones-matmul, mm2 attn^T-weighted v, normalize
  5. final projection out^T = w_p x attn_out^T + b_p
"""

import os
import sys
import threading

for _p in ("/opt/trn_rl_repo",):
    if _p not in sys.path and os.path.isdir(_p):
        sys.path.insert(0, _p)

import numpy as np

import concourse.bass as bass
import concourse.bacc as bacc
import concourse.mybir as mybir
import concourse.tile as tile

B, N, C, H, HD = 8, 512, 1024, 16, 64
P = 128
NP = C // P        # 8 feature-dim partition tiles
NT = N // P        # 4 token-dim partition tiles
NH2 = H // 2       # head pairs (two heads share one 128-partition tile)
SQRT_HD = 8.0      # sqrt(HD)
F32 = mybir.dt.float32
F32R = mybir.dt.float32r
AF = mybir.ActivationFunctionType
N_CORES = 8


def build_bass(reps=1):
    nc = bacc.Bacc("TRN2", target_bir_lowering=False, debug=False,
                   num_devices=N_CORES)

    xT = nc.dram_tensor("xT", [C, N], F32R, kind="ExternalInput").ap()
    muT = nc.dram_tensor("muT", [C, N], F32, kind="ExternalInput").ap()
    lvT = nc.dram_tensor("lvT", [C, N], F32, kind="ExternalInput").ap()
    al = nc.dram_tensor("alpha_r", [P, NT], F32, kind="ExternalInput").ap()
    wq = nc.dram_tensor("wqT", [C, C], F32R, kind="ExternalInput").ap()
    wk = nc.dram_tensor("wkT", [C, C], F32R, kind="ExternalInput").ap()
    wv = nc.dram_tensor("wvT8", [C, C], F32R, kind="ExternalInput").ap()
    wp = nc.dram_tensor("wpT", [C, C], F32R, kind="ExternalInput").ap()
    bp = nc.dram_tensor("bp_r", [P, NP], F32, kind="ExternalInput").ap()
    attn_o = nc.dram_tensor("attn_t", [H, N, N], F32, kind="ExternalOutput").ap()
    out_o = nc.dram_tensor("out_t", [C, N], F32, kind="ExternalOutput").ap()

    with tile.TileContext(nc) as tc:
        with (
            tc.tile_pool(name="persist", bufs=1) as pp,
            tc.tile_pool(name="acts", bufs=1) as ac,
            tc.tile_pool(name="wpool", bufs=3) as wpl,
            tc.tile_pool(name="expp", bufs=8) as ep,
            tc.tile_pool(name="attn_sb", bufs=4) as asb,
            tc.tile_pool(name="bcast", bufs=4) as bcp,
            tc.tile_pool(name="small", bufs=4) as sp,
            tc.tile_pool(name="fin", bufs=3) as fpl,
            tc.tile_pool(name="ps", bufs=8, space="PSUM") as ps,
        ):
          for _rep in range(reps):
            # constants + per-key bias inputs
            ones_f = pp.tile([P, 1], F32, tag="ones_f")
            nc.vector.memset(ones_f, 1.0)
            ones = pp.tile([P, 1], F32R, tag="ones")
            nc.vector.tensor_copy(ones, ones_f)
            ones_row_f = pp.tile([1, P], F32, tag="ones_row_f")
            nc.vector.memset(ones_row_f, 1.0)
            ones_row = pp.tile([1, P], F32R, tag="ones_row")
            nc.vector.tensor_copy(ones_row, ones_row_f)
            halves = pp.tile([P, 1], F32, tag="halves")
            nc.vector.memset(halves, 0.5)
            alpha_t = pp.tile([P, NT], F32, tag="alpha")
            nc.sync.dma_start(out=alpha_t, in_=al)
            lal = pp.tile([P, NT], F32, tag="lal")
            nc.scalar.activation(lal, alpha_t, AF.Ln)
            bias4 = pp.tile([P, NT], F32, tag="bias4")
            bp_t = pp.tile([P, NP], F32, tag="bp")
            nc.sync.dma_start(out=bp_t, in_=bp)

            # ---- phase E: elementwise NVIB transforms (transposed layout) ----
            mu_t, kin_t, x_t = [], [], []
            for i in range(NP):
                sl = slice(i * P, (i + 1) * P)
                lv = ac.tile([P, N], F32, tag=f"lv{i}")
                nc.sync.dma_start(out=lv, in_=lvT[sl, :])
                mu = ac.tile([P, N], F32, tag=f"mu{i}")
                nc.sync.dma_start(out=mu, in_=muT[sl, :])
                x = ac.tile([P, N], F32R, tag=f"x{i}")
                nc.sync.dma_start(out=x, in_=xT[sl, :])
                # lv <- 1 / (exp(lv) + sqrt_hd)   (biased variance reciprocal)
                nc.scalar.activation(lv, lv, AF.Exp)
                nc.vector.tensor_scalar_add(lv, lv, SQRT_HD)
                nc.vector.reciprocal(lv, lv)
                kin = ac.tile([P, N], F32R, tag=f"kin{i}")
                nc.vector.tensor_mul(kin, mu, lv)
                # mu <- mu * kin = mu^2 / biased_var   (l2 integrand)
                nc.vector.tensor_mul(mu, mu, kin.bitcast(F32))
                mu_t.append(mu)
                kin_t.append(kin)
                x_t.append(x)

            # ---- l2 per key + softmax key bias (full fp32 matmuls, N=1) ----
            for kc in range(NT):
                pl2 = ps.tile([P, 1], F32, tag="ps")
                for i in range(NP):
                    nc.tensor.matmul(pl2, mu_t[i][:, kc * P:(kc + 1) * P], halves,
                                     start=(i == 0), stop=(i == NP - 1))
                nc.vector.tensor_sub(bias4[:, kc:kc + 1], lal[:, kc:kc + 1], pl2)

            # ---- projections ----
            def project_T(wdram, rhs_tiles, name):
                """out^T tiles (c_out on partitions): lhsT = W^T k-slices."""
                psums = [ps.tile([P, N], F32, tag="ps", name=f"psum_{name}{m}")
                         for m in range(NP)]
                for k in range(NP):
                    wt = wpl.tile([P, C], F32R, tag="w")
                    nc.sync.dma_start(out=wt, in_=wdram[k * P:(k + 1) * P, :])
                    for m in range(NP):
                        nc.tensor.matmul(
                            psums[m],
                            wt[:, m * P:(m + 1) * P],
                            rhs_tiles[k],
                            start=(k == 0), stop=(k == NP - 1))
                outs = []
                for m in range(NP):
                    o = pp.tile([P, N], F32R, tag=f"{name}{m}")
                    nc.vector.tensor_copy(o, psums[m])
                    outs.append(o)
                return outs

            qT_t = project_T(wq, x_t, "qT")
            kT_t = project_T(wk, kin_t, "kT")

            # v in natural layout (tokens on partitions): lhsT = kin^T slices
            v_t = [pp.tile([P, C], F32R, tag=f"v{t}", name=f"v{t}")
                   for t in range(NT)]
            psv = [[ps.tile([P, N], F32, tag="ps", name=f"psv{t}_{jj}")
                    for jj in range(2)] for t in range(NT)]
            for k in range(NP):
                wt = wpl.tile([P, C], F32R, tag="w")
                nc.sync.dma_start(out=wt, in_=wv[k * P:(k + 1) * P, :])
                for t in range(NT):
                    for j in range(2):
                        nc.tensor.matmul(
                            psv[t][j],
                            kin_t[k][:, t * P:(t + 1) * P],
                            wt[:, j * N:(j + 1) * N],
                            start=(k == 0), stop=(k == NP - 1))
            for t in range(NT):
                for j in range(2):
                    nc.vector.tensor_copy(v_t[t][:, j * N:(j + 1) * N], psv[t][j])

            # ---- attention, one head pair per 128-partition tile ----
            out_t_tiles = []
            for j in range(NH2):
                exp_tiles = [[None] * NT, [None] * NT]
                bc = [None, None]
                ps_o = [None, None]
                # mm1 both heads first so PE stays busy while ACT does exps
                for e in range(2):
                    off = 64 * e
                    qh = qT_t[j][off:off + 64, :]
                    for kc in range(NT):
                        psl = ps.tile([P, N], F32, tag="ps",
                                      name=f"psl{j}_{e}_{kc}")
                        nc.tensor.matmul(
                            psl,
                            kT_t[j][off:off + 64, kc * P:(kc + 1) * P],
                            qh, start=True, stop=True)
                        et = ep.tile([P, N], F32R, tag="exp",
                                     name=f"et{j}_{e}_{kc}")
                        nc.scalar.activation(et, psl, AF.Exp,
                                             bias=bias4[:, kc:kc + 1], scale=1.0)
                        exp_tiles[e][kc] = et
                for e in range(2):
                    h = 2 * j + e
                    ps_d = ps.tile([1, N], F32, tag="ps", name=f"psd{j}_{e}")
                    for kc in range(NT):
                        nc.tensor.matmul(ps_d, ones,
                                         exp_tiles[e][kc],
                                         start=(kc == 0), stop=(kc == NT - 1))
                    po = ps.tile([64, N], F32, tag="ps", name=f"pso{j}_{e}")
                    for kc in range(NT):
                        nc.tensor.matmul(
                            po,
                            v_t[kc][:, h * HD:(h + 1) * HD],
                            exp_tiles[e][kc],
                            start=(kc == 0), stop=(kc == NT - 1))
                    ps_o[e] = po
                    rec = sp.tile([1, N], F32R, tag="rec", name=f"rec{j}_{e}")
                    with nc.allow_low_precision(reason="fp32r for PE broadcast"):
                        nc.vector.reciprocal(rec, ps_d)
                    # broadcast recip row to 128 partitions: ones_row outer rec
                    bc_ps = ps.tile([P, N], F32, tag="ps", name=f"bcps{j}_{e}")
                    nc.tensor.matmul(bc_ps, ones_row, rec, start=True, stop=True)
                    bc[e] = bcp.tile([P, N], F32, tag="bc", name=f"bc{j}_{e}")
                    nc.vector.tensor_copy(bc[e], bc_ps)
                    for kc in range(NT):
                        at = asb.tile([P, N], F32, tag="at",
                                      name=f"at{j}_{e}_{kc}")
                        nc.vector.tensor_mul(at, exp_tiles[e][kc].bitcast(F32),
                                             bc[e])
                        nc.sync.dma_start(
                            out=attn_o[h, kc * P:(kc + 1) * P, :], in_=at)
                ot = pp.tile([P, N], F32R, tag=f"ot{j}")
                nc.vector.tensor_mul(ot[0:64, :], ps_o[0][:, :], bc[0][0:64, :])
                oth = sp.tile([64, N], F32R, tag="oth", name=f"oth{j}")
                nc.vector.tensor_mul(oth, ps_o[1][:, :], bc[1][0:64, :])
                nc.sync.dma_start(out=ot[64:128, :], in_=oth)
                out_t_tiles.append(ot)

            # ---- final projection out^T = w_p @ attn_out^T + b_p ----
            psf = [ps.tile([P, N], F32, tag="ps", name=f"psf{m}")
                   for m in range(NP)]
            for k in range(NP):
                wt = wpl.tile([P, C], F32R, tag="w")
                nc.sync.dma_start(out=wt, in_=wp[k * P:(k + 1) * P, :])
                for m in range(NP):
                    # rhs: k-th 128-row slice of attn_out^T == head-pair tile k
                    nc.tensor.matmul(
                        psf[m],
                        wt[:, m * P:(m + 1) * P],
                        out_t_tiles[k],
                        start=(k == 0), stop=(k == NP - 1))
            for m in range(NP):
                fo = fpl.tile([P, N], F32, tag="fo")
                nc.vector.tensor_scalar_add(fo, psf[m], bp_t[:, m:m + 1])
                nc.sync.dma_start(out=out_o[m * P:(m + 1) * P, :], in_=fo)

    nc.compile()
    return nc


def shard_inputs(x, mu, logvar, alpha, w_q, w_k, w_v, w_p, b_p):
    """Host-side data prep: transpose + reshape, one in_map per core."""
    f = np.float32
    wqT = np.ascontiguousarray(np.asarray(w_q, f).T)
    wkT = np.ascontiguousarray(np.asarray(w_k, f).T)
    wvT8 = np.ascontiguousarray(np.asarray(w_v, f).T * SQRT_HD)
    wpT = np.ascontiguousarray(np.asarray(w_p, f).T)
    bp_r = np.ascontiguousarray(np.asarray(b_p, f).reshape(NP, P).T)
    in_maps = []
    for b in range(B):
        a = np.asarray(alpha[b], f).reshape(N)
        in_maps.append({
            "xT": np.ascontiguousarray(np.asarray(x[b], f).T),
            "muT": np.ascontiguousarray(np.asarray(mu[b], f).T),
            "lvT": np.ascontiguousarray(np.asarray(logvar[b], f).T),
            "alpha_r": np.ascontiguousarray(a.reshape(NT, P).T),
            "wqT": wqT, "wkT": wkT, "wvT8": wvT8, "wpT": wpT, "bp_r": bp_r,
        })
    return in_maps


def unshard_outputs(results):
    """results: list of per-core {attn_t, out_t} -> (out, attn) full arrays."""
    out = np.empty((B, N, C), np.float32)
    attn = np.empty((B, H, N, N), np.float32)
    for b, r in enumerate(results):
        out[b] = r["out_t"].T
        attn[b] = r["attn_t"].transpose(0, 2, 1)
    return out, attn


_RUNNER_LOCK = threading.Lock()
_RUNNER = None


def _get_runner(reps=1):
    """Build the Bass module + jitted SPMD executable once; reuse across calls."""
    global _RUNNER
    with _RUNNER_LOCK:
        if _RUNNER is None:
            _RUNNER = {}
        if reps in _RUNNER:
            return _RUNNER[reps]

        nc = build_bass(reps)

        import jax
        from jax.sharding import Mesh, PartitionSpec
        from jax.experimental.shard_map import shard_map
        from concourse import bass2jax
        import concourse.mybir as mb

        bass2jax.install_neuronx_cc_hook()

        partition_name = (nc.partition_id_tensor.name
                          if nc.partition_id_tensor else None)
        in_names, out_names, out_avals, zero_shapes = [], [], [], []
        for alloc in nc.m.functions[0].allocations:
            if not isinstance(alloc, mb.MemoryLocationSet):
                continue
            name = alloc.memorylocations[0].name
            if alloc.kind == "ExternalInput":
                if name != partition_name:
                    in_names.append(name)
            elif alloc.kind == "ExternalOutput":
                shape = tuple(alloc.tensor_shape)
                dtype = mb.dt.np(alloc.dtype)
                out_names.append(name)
                out_avals.append(jax.core.ShapedArray(shape, dtype))
                zero_shapes.append((shape, dtype))
        n_params = len(in_names)
        all_names = in_names + out_names
        if partition_name is not None:
            all_names.append(partition_name)
        donate = tuple(range(n_params, n_params + len(out_names)))

        def _body(*args):
            operands = list(args)
            if partition_name is not None:
                operands.append(bass2jax.partition_id_tensor())
            outs = bass2jax._bass_exec_p.bind(
                *operands,
                out_avals=tuple(out_avals),
                in_names=tuple(all_names),
                out_names=tuple(out_names),
                lowering_input_output_aliases=(),
                sim_require_finite=True,
                sim_require_nnan=True,
                nc=nc,
            )
            return tuple(outs)

        devices = jax.devices()[:N_CORES]
        mesh = Mesh(np.asarray(devices), ("core",))
        in_specs = (PartitionSpec("core"),) * (n_params + len(out_names))
        out_specs = (PartitionSpec("core"),) * len(out_names)
        sharded = jax.jit(
            shard_map(_body, mesh=mesh, in_specs=in_specs,
                      out_specs=out_specs, check_rep=False),
            donate_argnums=donate, keep_unused=True)

        class Runner:
            def concat_inputs(self, in_maps):
                per_core = [[np.asarray(m[n]) for n in in_names] for m in in_maps]
                return [
                    np.concatenate([per_core[c][i] for c in range(N_CORES)],
                                   axis=0)
                    for i in range(n_params)
                ]

            def concat_zeros(self):
                return [np.zeros((N_CORES * s[0], *s[1:]), d)
                        for (s, d) in zero_shapes]

            def split_outputs(self, out_arrs):
                out_arrs = [np.asarray(a) for a in out_arrs]
                return [
                    {name: out_arrs[i].reshape(N_CORES, *zero_shapes[i][0])[c]
                     for i, name in enumerate(out_names)}
                    for c in range(N_CORES)
                ]

            def execute(self, in_maps):
                out_arrs = sharded(*self.concat_inputs(in_maps),
                                   *self.concat_zeros())
                return self.split_outputs(out_arrs)

        r = Runner()
        r.body = _body
        r.n_params = n_params
        r.donate = donate
        r.in_specs = in_specs
        r.out_specs = out_specs
        r.sharded = sharded
        r.mesh = mesh
        r.in_names = in_names
        r.out_names = out_names
        r.zero_shapes = zero_shapes
        _RUNNER[reps] = r
        return _RUNNER[reps]


def kernel(x, mu, logvar, alpha, pi, z, w_q, w_k, w_v, w_p, b_p):
    in_maps = shard_inputs(x, mu, logvar, alpha, w_q, w_k, w_v, w_p, b_p)
    results = _get_runner(1).execute(in_maps)
    return unshard_outputs(results)
